# revision 24
# baseline (speedup 1.0000x reference)
"""Multi-head attention with bias, distributed over 8 trn2 NeuronCores.

Reference computation (per batch b):
    q = (x @ Wq.T) * depth**-0.5 ; k = y @ Wk.T ; v = y @ Wv.T     (per-head split)
    out = softmax(q @ k.T + bias) @ v @ Wo.T

Sharding: 8 cores = 4 batches x 2 query-row halves.  Core c handles batch
b = c//2 and query rows (c%2)*1024 .. +1024.  k/v projections are computed
redundantly inside each pair (25% extra flops) so there are NO collectives.

Device-side layout (everything "transposed", feature dim on partitions):
    qT/kT = W.T-projected activations [d_out, seq]; v natural [seq, d_out].
    logitsT[kk, i] = kT_h-slice.T @ qT_h-slice  (K=64 contraction)
    expw = exp(logitsT) * exp(bias).T           (exp(bias) precomputed on host)
    attnT_h(+denom row) = [v_h | ones].T @ expw (K=128, denom rides as row 64)
    normalize via batched DVE reciprocal + DMA partition-broadcast from DRAM
    outT = Wo.T-proj of normalized attnT.
Host does: transposes, bf16 casts, exp(bias), scale fold into Wq.
"""

import numpy as np
import ml_dtypes
from contextlib import ExitStack

import concourse.bass as bass
import concourse.mybir as mybir
import concourse.tile as tile
from concourse import bacc
from concourse.bass_utils import run_bass_kernel_spmd

# full-problem dims (hardcoded per spec)
B, S, D, H = 4, 2048, 1024, 16
DEPTH = D // H            # 64
P = 128
NCORES = 8

BF = mybir.dt.bfloat16
F32 = mybir.dt.float32
EXP = mybir.ActivationFunctionType.Exp

TRACE = False
last_exec_time_ns = None
last_results = None


def _chunks(total, step):
    return [(n0, min(n0 + step, total)) for n0 in range(0, total, step)]


def _attn_body(ctx, tc, io, S_, D_, H_, SL_):
    """Emit the per-core kernel.  S_: kv seq len, SL_: q rows on this core.

    Software-pipelined emission: the PE instruction stream interleaves
    v/q/k projection matmuls into the ACT-bound attention inner loop so
    the ScalarE exp stream (the bottleneck) starts early and never
    starves.  Normalization is per head-pair so only the last pair's
    reciprocal chain sits in the tail.
    """
    nc = tc.nc
    NT = D_ // P              # d tiles
    KT = S_ // P              # kk tiles
    HPT = P // DEPTH          # heads per d-tile = 2
    xT, yT, ebT, wqT, wkT, wvT, woT, outT = (
        io[k] for k in ("xT", "yT", "ebT", "wqT", "wkT", "wvT", "woT", "outT"))

    ebpool = ctx.enter_context(tc.tile_pool(name="ebpool", bufs=KT))
    qpool = ctx.enter_context(tc.tile_pool(name="qpool", bufs=2))
    kpool = ctx.enter_context(tc.tile_pool(name="kpool", bufs=2))
    vpool = ctx.enter_context(tc.tile_pool(name="vpool", bufs=KT))
    epool = ctx.enter_context(tc.tile_pool(name="epool", bufs=4))
    stpool = ctx.enter_context(tc.tile_pool(name="stpool", bufs=4))
    smpool = ctx.enter_context(tc.tile_pool(name="smpool", bufs=2))
    plp = ctx.enter_context(tc.tile_pool(name="plp", bufs=2, space="PSUM"))
    pap = ctx.enter_context(tc.tile_pool(name="pap", bufs=2, space="PSUM"))
    dpool = ctx.enter_context(tc.tile_pool(name="dpool", bufs=1, space="DRAM"))

    v_sb = [vpool.tile([P, H_, 66], BF, tag="v66", name=f"v{c}", bufs=KT)
            for c in range(KT)]
    rscr = dpool.tile([H_, SL_], BF, tag="rscr", name="rscr", bufs=1)
    audram = dpool.tile([D_, SL_], BF, tag="audram", name="audram", bufs=1)

    with tc.tile_pool(name="ypool", bufs=NT) as ypool, \
         tc.tile_pool(name="xpool", bufs=NT) as xpool, \
         tc.tile_pool(name="wqpool", bufs=NT) as wqpool, \
         tc.tile_pool(name="wvpool", bufs=NT) as wvpool, \
         tc.tile_pool(name="wkpool", bufs=NT) as wkpool:
        x_sb = [xpool.tile([P, SL_], BF, tag="xT", name=f"x{t}", bufs=NT)
                for t in range(NT)]
        for t in range(NT):
            nc.sync.dma_start(out=x_sb[t], in_=xT[t * P:(t + 1) * P, :])
        wq_sb = [wqpool.tile([P, D_], BF, tag="wq", name=f"wq{t}", bufs=NT)
                 for t in range(NT)]
        for t in range(NT):
            nc.sync.dma_start(out=wq_sb[t], in_=wqT[t * P:(t + 1) * P, :])
        y_sb = [ypool.tile([P, S_], BF, tag="yT", name=f"y{t}", bufs=NT)
                for t in range(NT)]
        for t in range(NT):
            nc.sync.dma_start(out=y_sb[t], in_=yT[t * P:(t + 1) * P, :])
        wv_sb = [wvpool.tile([P, D_], BF, tag="wv", name=f"wv{t}", bufs=NT)
                 for t in range(NT)]
        for t in range(NT):
            nc.sync.dma_start(out=wv_sb[t], in_=wvT[t * P:(t + 1) * P, :])
        wk_sb = [wkpool.tile([P, D_], BF, tag="wk", name=f"wk{t}", bufs=NT)
                 for t in range(NT)]
        for t in range(NT):
            nc.sync.dma_start(out=wk_sb[t], in_=wkT[t * P:(t + 1) * P, :])
        eb_sb = [ebpool.tile([P, SL_], BF, tag="eb", name=f"eb{c}", bufs=KT)
                 for c in range(KT)]
        for c in range(KT):
            nc.sync.dma_start(out=eb_sb[c], in_=ebT[c * P:(c + 1) * P, :])

        # warm-up heartbeats: tiny matmuls chained to arriving input DMAs
        # keep the PE HAM activity window alive through the load phase so
        # the first projections run at 2.4 GHz
        wj0 = min(512, SL_)
        jnk0 = plp.tile([1, 1024], F32, tag="pl", name="jnk0", bufs=2)
        for t in range(NT):
            nc.tensor.matmul(jnk0[0:1, 0:wj0], lhsT=x_sb[t][0:1, 0:1],
                             rhs=x_sb[t][0:1, 0:wj0], start=True, stop=True)
            nc.tensor.matmul(jnk0[0:1, 0:wj0], lhsT=y_sb[t][0:1, 0:1],
                             rhs=y_sb[t][0:1, 0:wj0], start=True, stop=True)

        # ---- emission helpers (deferred work units for pipelining) ----
        def emit_v_tile(c):
            vt = v_sb[c]
            nc.vector.memset(vt[:, :, 64:65], 1.0)
            nc.vector.memset(vt[:, :, 65:66], 0.0)
            for gi, (n0, n1) in enumerate(_chunks(D_, 512)):
                ps = plp.tile([P, 1024], F32, tag="pl", name=f"psv{c}_{gi}",
                              bufs=2)
                for u in range(NT):
                    nc.tensor.matmul(ps[:, 0:n1 - n0],
                                     lhsT=y_sb[u][:, c * P:(c + 1) * P],
                                     rhs=wv_sb[u][:, n0:n1],
                                     start=(u == 0), stop=(u == NT - 1))
                ng = (n1 - n0) // DEPTH
                src = ps[:, 0:n1 - n0].rearrange("p (g d) -> p g d", d=DEPTH)
                nc.vector.tensor_copy(vt[:, gi * ng:(gi + 1) * ng, 0:DEPTH],
                                      src)

        def emit_q_group(qt, t, n0, n1):
            ps = plp.tile([P, 1024], F32, tag="pl", name=f"psq{t}_{n0}",
                          bufs=2)
            for u in range(NT):
                nc.tensor.matmul(ps[:, 0:n1 - n0],
                                 lhsT=wq_sb[u][:, t * P:(t + 1) * P],
                                 rhs=x_sb[u][:, n0:n1],
                                 start=(u == 0), stop=(u == NT - 1))
            nc.vector.tensor_copy(qt[:, n0:n1], ps[:, 0:n1 - n0])

        def emit_k_group(kt, t, n0, n1):
            ps = plp.tile([P, 1024], F32, tag="pl", name=f"psk{t}_{n0}",
                          bufs=2)
            for u in range(NT):
                nc.tensor.matmul(ps[:, 0:n1 - n0],
                                 lhsT=wk_sb[u][:, t * P:(t + 1) * P],
                                 rhs=y_sb[u][:, n0:n1],
                                 start=(u == 0), stop=(u == NT - 1))
            nc.vector.tensor_copy(kt[:, n0:n1], ps[:, 0:n1 - n0])

        def proj_thunks(qt, kt, t):
            return ([lambda n0=n0, n1=n1: emit_q_group(qt, t, n0, n1)
                     for n0, n1 in _chunks(SL_, 512)] +
                    [lambda n0=n0, n1=n1: emit_k_group(kt, t, n0, n1)
                     for n0, n1 in _chunks(S_, 512)])

        # ---- prologue: q0/k0 projection, then first v tiles ----
        q_cur = qpool.tile([P, SL_], BF, tag="qT", name="q0", bufs=2)
        k_cur = kpool.tile([P, S_], BF, tag="kT", name="k0", bufs=2)
        for th in proj_thunks(q_cur, k_cur, 0):
            th()
        vlead = 1
        for c in range(vlead):
            emit_v_tile(c)

        for t in range(NT):
            ha, hb = HPT * t, HPT * t + 1
            pattn = [pap.tile([65, SL_], F32, tag="pattn",
                              name=f"pa{ha + hf}", bufs=2)
                     for hf in range(HPT)]
            # deferred emissions spread across this pair's c-loop
            thunks = []
            if t == 0:
                thunks += [lambda c=c: emit_v_tile(c)
                           for c in range(vlead, KT)]
            if t + 1 < NT:
                q_nxt = qpool.tile([P, SL_], BF, tag="qT", name=f"q{t + 1}",
                                   bufs=2)
                k_nxt = kpool.tile([P, S_], BF, tag="kT", name=f"k{t + 1}",
                                   bufs=2)
                thunks += proj_thunks(q_nxt, k_nxt, t + 1)
            # schedule thunk i after c-iteration floor(i * KT / len); for
            # pairs with only proj thunks, delay by 2 iterations so the
            # QK->exp pipeline fills before the PE detours into projections
            sched = {}
            off = 0 if t == 0 else min(2, KT - 1)
            span_c = KT - off
            for i, th in enumerate(thunks):
                sched.setdefault(off + i * span_c // max(1, len(thunks)),
                                 []).append(th)

            for c in range(KT):
                for n0, n1 in _chunks(SL_, 512):
                    w = n1 - n0
                    plt = plp.tile([P, 1024], F32, tag="pl",
                                   name=f"pl{ha}_{c}_{n0}", bufs=2)
                    nc.tensor.matmul(plt[:, 0:w],
                                     lhsT=k_cur[0:DEPTH, c * P:(c + 1) * P],
                                     rhs=q_cur[0:DEPTH, n0:n1],
                                     start=True, stop=True)
                    nc.tensor.matmul(plt[:, w:2 * w],
                                     lhsT=k_cur[DEPTH:2 * DEPTH,
                                                c * P:(c + 1) * P],
                                     rhs=q_cur[DEPTH:2 * DEPTH, n0:n1],
                                     start=True, stop=True)
                    ew = epool.tile([P, 1024], BF, tag="ew",
                                    name=f"ew{ha}_{c}_{n0}", bufs=2)
                    nc.scalar.activation(ew[:, 0:2 * w], plt[:, 0:2 * w], EXP)
                    ew2 = epool.tile([P, 1024], BF, tag="ew2",
                                     name=f"ew2{ha}_{c}_{n0}", bufs=2)
                    nc.vector.tensor_mul(ew2[:, 0:w], ew[:, 0:w],
                                         eb_sb[c][:, n0:n1])
                    nc.vector.tensor_mul(ew2[:, w:2 * w], ew[:, w:2 * w],
                                         eb_sb[c][:, n0:n1])
                    nc.tensor.matmul(pattn[0][:, n0:n1],
                                     lhsT=v_sb[c][:, ha, 0:65],
                                     rhs=ew2[:, 0:w],
                                     start=(c == 0), stop=(c == KT - 1))
                    nc.tensor.matmul(pattn[1][:, n0:n1],
                                     lhsT=v_sb[c][:, hb, 0:65],
                                     rhs=ew2[:, w:2 * w],
                                     start=(c == 0), stop=(c == KT - 1))
                for th in sched.get(c, ()):
                    th()

            # ---- epilogue + per-pair normalization ----
            # sau rows 0-63: unnormalized attn (base partition 0); row 64:
            # denominator.  Normalize in SBUF then bounce the finished rows
            # to DRAM so no [128,SL] attn tiles stay resident.
            den_t = smpool.tile([HPT, SL_], BF, tag="dent", name=f"den{t}",
                                bufs=2)
            saus = []
            for hf in range(HPT):
                h = ha + hf
                sau = stpool.tile([65, SL_], BF, tag="sau", name=f"sa{h}",
                                  bufs=3)
                saus.append(sau)
                nc.vector.tensor_copy(sau, pattn[hf])
                nc.sync.dma_start(out=den_t[hf:hf + 1, :], in_=sau[64:65, :])
            wj = min(512, SL_)
            jnk = None
            if t == NT - 1:
                jnk = plp.tile([1, 1024], F32, tag="pl", name="jnk", bufs=2)

            def beat(ap):
                # tiny dependent matmul: keeps the PE HAM activity window
                # alive across the serial normalize tail (else the output
                # projection starts at the 1.2 GHz throttled clock)
                if jnk is not None:
                    nc.tensor.matmul(jnk[0:1, 0:wj], lhsT=ap[0:1, 0:1],
                                     rhs=ap[0:1, 0:wj], start=True, stop=True)

            denf = smpool.tile([HPT, SL_], F32, tag="denf", name=f"dnf{t}",
                               bufs=1)
            nc.vector.tensor_copy(denf, den_t)
            beat(denf)
            recipf = smpool.tile([HPT, SL_], F32, tag="recipf",
                                 name=f"rcf{t}", bufs=1)
            nc.vector.reciprocal_approx_fast(recipf, denf)
            beat(recipf)
            recipb = smpool.tile([HPT, SL_], BF, tag="recipb",
                                 name=f"rcb{t}", bufs=1)
            nc.vector.tensor_copy(recipb, recipf)
            nc.sync.dma_start(out=rscr[HPT * t:HPT * (t + 1), :], in_=recipb)
            for hf in range(HPT):
                h = ha + hf
                bc = smpool.tile([DEPTH, SL_], BF, tag="bc", name=f"bc{h}",
                                 bufs=1)
                nc.sync.dma_start(
                    out=bc, in_=rscr[h:h + 1, :].partition_broadcast(DEPTH))
                beat(bc)
                anh = smpool.tile([DEPTH, SL_], BF, tag="anh", name=f"an{h}",
                                  bufs=2)
                nc.vector.tensor_mul(anh, saus[hf][0:64, :], bc)
                beat(anh)
                nc.sync.dma_start(
                    out=audram[t * P + hf * DEPTH:t * P + (hf + 1) * DEPTH, :],
                    in_=anh)
            if t + 1 < NT:
                q_cur, k_cur = q_nxt, k_nxt

    # ---------------- output projection ----------------
    opool = ctx.enter_context(tc.tile_pool(name="opool", bufs=2))
    wopool = ctx.enter_context(tc.tile_pool(name="wopool", bufs=NT))
    ropool = ctx.enter_context(tc.tile_pool(name="ropool", bufs=NT))
    wo_sb = [wopool.tile([P, D_], BF, tag="wo", name=f"wo{t}", bufs=NT)
             for t in range(NT)]
    for t in range(NT):
        nc.gpsimd.dma_start(out=wo_sb[t], in_=woT[t * P:(t + 1) * P, :])
    an_sb = [ropool.tile([P, SL_], BF, tag="an", name=f"ran{t}", bufs=NT)
             for t in range(NT)]
    for t in range(NT):
        nc.gpsimd.dma_start(out=an_sb[t], in_=audram[t * P:(t + 1) * P, :])
    jnk2 = plp.tile([1, 1024], F32, tag="pl", name="jnk2", bufs=2)
    wj = min(512, SL_)
    for t in (0, NT - 1):
        nc.tensor.matmul(jnk2[0:1, 0:wj], lhsT=an_sb[t][0:1, 0:1],
                         rhs=an_sb[t][0:1, 0:wj], start=True, stop=True)
    for m in range(NT):
        osb = opool.tile([P, SL_], F32, tag="osb", name=f"o{m}", bufs=2)
        for n0, n1 in _chunks(SL_, 512):
            ps = plp.tile([P, 1024], F32, tag="pl", name=f"pso{m}_{n0}", bufs=2)
            for t in range(NT):
                nc.tensor.matmul(ps[:, 0:n1 - n0],
                                 lhsT=wo_sb[t][:, m * P:(m + 1) * P],
                                 rhs=an_sb[t][:, n0:n1],
                                 start=(t == 0), stop=(t == NT - 1))
            nc.vector.tensor_copy(osb[:, n0:n1], ps[:, 0:n1 - n0])
        nc.sync.dma_start(out=outT[m * P:(m + 1) * P, :], in_=osb)


def build_nc(S_=S, D_=D, H_=H, SL_=None):
    if SL_ is None:
        SL_ = S_ // 2
    nc = bacc.Bacc("TRN2", target_bir_lowering=False, debug=False)
    io = {
        "xT": nc.dram_tensor("xT", [D_, SL_], BF, kind="ExternalInput").ap(),
        "yT": nc.dram_tensor("yT", [D_, S_], BF, kind="ExternalInput").ap(),
        "ebT": nc.dram_tensor("ebT", [S_, SL_], BF, kind="ExternalInput").ap(),
        "wqT": nc.dram_tensor("wqT", [D_, D_], BF, kind="ExternalInput").ap(),
        "wkT": nc.dram_tensor("wkT", [D_, D_], BF, kind="ExternalInput").ap(),
        "wvT": nc.dram_tensor("wvT", [D_, D_], BF, kind="ExternalInput").ap(),
        "woT": nc.dram_tensor("woT", [D_, D_], BF, kind="ExternalInput").ap(),
        "outT": nc.dram_tensor("outT", [D_, SL_], F32,
                               kind="ExternalOutput").ap(),
    }
    with tile.TileContext(nc) as tc:
        with ExitStack() as ctx:
            _attn_body(ctx, tc, io, S_, D_, H_, SL_)
    nc.compile()
    return nc


_NC_CACHE = None


def kernel(x, y, bias, Wq, Wk, Wv, Wo):
    global _NC_CACHE, last_exec_time_ns, last_results
    x = np.asarray(x, np.float32)
    y = np.asarray(y, np.float32)
    bias = np.asarray(bias, np.float32)
    Wq, Wk, Wv, Wo = (np.asarray(w, np.float32) for w in (Wq, Wk, Wv, Wo))
    SL_ = S // 2
    if _NC_CACHE is None:
        _NC_CACHE = build_nc()
    nc = _NC_CACHE

    bf = ml_dtypes.bfloat16
    scale = DEPTH ** -0.5
    wqT = np.ascontiguousarray(Wq.T * scale).astype(bf)
    wkT = np.ascontiguousarray(Wk.T).astype(bf)
    wvT = np.ascontiguousarray(Wv.T).astype(bf)
    woT = np.ascontiguousarray(Wo.T).astype(bf)
    eb = np.exp(bias[0, 0].astype(np.float32))
    ebT_half = [np.ascontiguousarray(eb[q0:q0 + SL_, :].T).astype(bf)
                for q0 in (0, SL_)]
    yT_all = [np.ascontiguousarray(y[b].T).astype(bf) for b in range(B)]

    in_maps = []
    for core in range(NCORES):
        b, half = divmod(core, 2)
        qs = half * SL_
        in_maps.append({
            "xT": np.ascontiguousarray(x[b, qs:qs + SL_, :].T).astype(bf),
            "yT": yT_all[b],
            "ebT": ebT_half[half],
            "wqT": wqT, "wkT": wkT, "wvT": wvT, "woT": woT,
        })

    res = run_bass_kernel_spmd(nc, in_maps, core_ids=list(range(NCORES)),
                               trace=TRACE)
    last_exec_time_ns = res.exec_time_ns
    last_results = res
    out = np.empty((B, S, D), np.float32)
    for core in range(NCORES):
        b, half = divmod(core, 2)
        qs = half * SL_
        out[b, qs:qs + SL_, :] = res.results[core]["outT"].T
    return out


# revision 25
# speedup vs baseline: 1.0162x; 1.0162x over previous
"""Multi-head attention with bias, distributed over 8 trn2 NeuronCores.

Reference computation (per batch b):
    q = (x @ Wq.T) * depth**-0.5 ; k = y @ Wk.T ; v = y @ Wv.T     (per-head split)
    out = softmax(q @ k.T + bias) @ v @ Wo.T

Sharding: 8 cores = 4 batches x 2 query-row halves.  Core c handles batch
b = c//2 and query rows (c%2)*1024 .. +1024.  k/v projections are computed
redundantly inside each pair (25% extra flops) so there are NO collectives.

Device-side layout (everything "transposed", feature dim on partitions):
    qT/kT = W.T-projected activations [d_out, seq]; v natural [seq, d_out].
    logitsT[kk, i] = kT_h-slice.T @ qT_h-slice  (K=64 contraction)
    expw = exp(logitsT) * exp(bias).T           (exp(bias) precomputed on host)
    attnT_h(+denom row) = [v_h | ones].T @ expw (K=128, denom rides as row 64)
    normalize via batched DVE reciprocal + DMA partition-broadcast from DRAM
    outT = Wo.T-proj of normalized attnT.
Host does: transposes, bf16 casts, exp(bias), scale fold into Wq.
"""

import numpy as np
import ml_dtypes
from contextlib import ExitStack

import concourse.bass as bass
import concourse.mybir as mybir
import concourse.tile as tile
from concourse import bacc
from concourse.bass_utils import run_bass_kernel_spmd

# full-problem dims (hardcoded per spec)
B, S, D, H = 4, 2048, 1024, 16
DEPTH = D // H            # 64
P = 128
NCORES = 8

BF = mybir.dt.bfloat16
F32 = mybir.dt.float32
EXP = mybir.ActivationFunctionType.Exp

TRACE = False
last_exec_time_ns = None
last_results = None


def _chunks(total, step):
    return [(n0, min(n0 + step, total)) for n0 in range(0, total, step)]


def _attn_body(ctx, tc, io, S_, D_, H_, SL_):
    """Emit the per-core kernel.  S_: kv seq len, SL_: q rows on this core.

    Software-pipelined emission: the PE instruction stream interleaves
    v/q/k projection matmuls into the ACT-bound attention inner loop so
    the ScalarE exp stream (the bottleneck) starts early and never
    starves.  Normalization is per head-pair so only the last pair's
    reciprocal chain sits in the tail.
    """
    nc = tc.nc
    NT = D_ // P              # d tiles
    KT = S_ // P              # kk tiles
    HPT = P // DEPTH          # heads per d-tile = 2
    xT, yT, ebT, wqT, wkT, wvT, woT, outT = (
        io[k] for k in ("xT", "yT", "ebT", "wqT", "wkT", "wvT", "woT", "outT"))

    ebpool = ctx.enter_context(tc.tile_pool(name="ebpool", bufs=KT))
    qpool = ctx.enter_context(tc.tile_pool(name="qpool", bufs=2))
    kpool = ctx.enter_context(tc.tile_pool(name="kpool", bufs=2))
    vpool = ctx.enter_context(tc.tile_pool(name="vpool", bufs=KT))
    epool = ctx.enter_context(tc.tile_pool(name="epool", bufs=4))
    stpool = ctx.enter_context(tc.tile_pool(name="stpool", bufs=4))
    smpool = ctx.enter_context(tc.tile_pool(name="smpool", bufs=2))
    plp = ctx.enter_context(tc.tile_pool(name="plp", bufs=2, space="PSUM"))
    pap = ctx.enter_context(tc.tile_pool(name="pap", bufs=2, space="PSUM"))
    dpool = ctx.enter_context(tc.tile_pool(name="dpool", bufs=1, space="DRAM"))

    v_sb = [vpool.tile([P, H_, 66], BF, tag="v66", name=f"v{c}", bufs=KT)
            for c in range(KT)]
    rscr = dpool.tile([H_, SL_], BF, tag="rscr", name="rscr", bufs=1)
    audram = dpool.tile([D_, SL_], BF, tag="audram", name="audram", bufs=1)

    with tc.tile_pool(name="ypool", bufs=NT) as ypool, \
         tc.tile_pool(name="xpool", bufs=NT) as xpool, \
         tc.tile_pool(name="wqpool", bufs=NT) as wqpool, \
         tc.tile_pool(name="wvpool", bufs=NT) as wvpool, \
         tc.tile_pool(name="wkpool", bufs=NT) as wkpool:
        x_sb = [xpool.tile([P, SL_], BF, tag="xT", name=f"x{t}", bufs=NT)
                for t in range(NT)]
        for t in range(NT):
            nc.sync.dma_start(out=x_sb[t], in_=xT[t * P:(t + 1) * P, :])
        wq_sb = [wqpool.tile([P, D_], BF, tag="wq", name=f"wq{t}", bufs=NT)
                 for t in range(NT)]
        for t in range(NT):
            nc.sync.dma_start(out=wq_sb[t], in_=wqT[t * P:(t + 1) * P, :])
        y_sb = [ypool.tile([P, S_], BF, tag="yT", name=f"y{t}", bufs=NT)
                for t in range(NT)]
        for t in range(NT):
            nc.sync.dma_start(out=y_sb[t], in_=yT[t * P:(t + 1) * P, :])
        wv_sb = [wvpool.tile([P, D_], BF, tag="wv", name=f"wv{t}", bufs=NT)
                 for t in range(NT)]
        for t in range(NT):
            nc.sync.dma_start(out=wv_sb[t], in_=wvT[t * P:(t + 1) * P, :])
        wk_sb = [wkpool.tile([P, D_], BF, tag="wk", name=f"wk{t}", bufs=NT)
                 for t in range(NT)]
        for t in range(NT):
            nc.sync.dma_start(out=wk_sb[t], in_=wkT[t * P:(t + 1) * P, :])
        eb_sb = [ebpool.tile([P, SL_], BF, tag="eb", name=f"eb{c}", bufs=KT)
                 for c in range(KT)]
        for c in range(KT):
            nc.sync.dma_start(out=eb_sb[c], in_=ebT[c * P:(c + 1) * P, :])

        # warm-up heartbeats: tiny matmuls chained to arriving input DMAs
        # keep the PE HAM activity window alive through the load phase so
        # the first projections run at 2.4 GHz
        wj0 = min(512, SL_)
        jnk0 = plp.tile([1, 1024], F32, tag="pl", name="jnk0", bufs=2)
        for t in range(NT):
            nc.tensor.matmul(jnk0[0:1, 0:wj0], lhsT=x_sb[t][0:1, 0:1],
                             rhs=x_sb[t][0:1, 0:wj0], start=True, stop=True)
            nc.tensor.matmul(jnk0[0:1, 0:wj0], lhsT=y_sb[t][0:1, 0:1],
                             rhs=y_sb[t][0:1, 0:wj0], start=True, stop=True)

        # ---- emission helpers (deferred work units for pipelining) ----
        def emit_v_tile(c):
            vt = v_sb[c]
            nc.vector.memset(vt[:, :, 64:65], 1.0)
            nc.vector.memset(vt[:, :, 65:66], 0.0)
            for gi, (n0, n1) in enumerate(_chunks(D_, 512)):
                ps = plp.tile([P, 1024], F32, tag="pl", name=f"psv{c}_{gi}",
                              bufs=2)
                for u in range(NT):
                    nc.tensor.matmul(ps[:, 0:n1 - n0],
                                     lhsT=y_sb[u][:, c * P:(c + 1) * P],
                                     rhs=wv_sb[u][:, n0:n1],
                                     start=(u == 0), stop=(u == NT - 1))
                ng = (n1 - n0) // DEPTH
                src = ps[:, 0:n1 - n0].rearrange("p (g d) -> p g d", d=DEPTH)
                nc.vector.tensor_copy(vt[:, gi * ng:(gi + 1) * ng, 0:DEPTH],
                                      src)

        def emit_q_group(qt, t, n0, n1):
            ps = plp.tile([P, 1024], F32, tag="pl", name=f"psq{t}_{n0}",
                          bufs=2)
            for u in range(NT):
                nc.tensor.matmul(ps[:, 0:n1 - n0],
                                 lhsT=wq_sb[u][:, t * P:(t + 1) * P],
                                 rhs=x_sb[u][:, n0:n1],
                                 start=(u == 0), stop=(u == NT - 1))
            nc.vector.tensor_copy(qt[:, n0:n1], ps[:, 0:n1 - n0])

        def emit_k_group(kt, t, n0, n1):
            ps = plp.tile([P, 1024], F32, tag="pl", name=f"psk{t}_{n0}",
                          bufs=2)
            for u in range(NT):
                nc.tensor.matmul(ps[:, 0:n1 - n0],
                                 lhsT=wk_sb[u][:, t * P:(t + 1) * P],
                                 rhs=y_sb[u][:, n0:n1],
                                 start=(u == 0), stop=(u == NT - 1))
            nc.vector.tensor_copy(kt[:, n0:n1], ps[:, 0:n1 - n0])

        def proj_thunks(qt, kt, t):
            return ([lambda n0=n0, n1=n1: emit_q_group(qt, t, n0, n1)
                     for n0, n1 in _chunks(SL_, 512)] +
                    [lambda n0=n0, n1=n1: emit_k_group(kt, t, n0, n1)
                     for n0, n1 in _chunks(S_, 512)])

        # ---- prologue: q0/k0 projection, then first v tiles ----
        q_cur = qpool.tile([P, SL_], BF, tag="qT", name="q0", bufs=2)
        k_cur = kpool.tile([P, S_], BF, tag="kT", name="k0", bufs=2)
        for th in proj_thunks(q_cur, k_cur, 0):
            th()
        vlead = min(2, KT)
        for c in range(vlead):
            emit_v_tile(c)

        for t in range(NT):
            ha, hb = HPT * t, HPT * t + 1
            pattn = [pap.tile([65, SL_], F32, tag="pattn",
                              name=f"pa{ha + hf}", bufs=2)
                     for hf in range(HPT)]
            # deferred emissions spread across this pair's c-loop
            thunks = []
            if t == 0:
                thunks += [lambda c=c: emit_v_tile(c)
                           for c in range(vlead, KT)]
            if t + 1 < NT:
                q_nxt = qpool.tile([P, SL_], BF, tag="qT", name=f"q{t + 1}",
                                   bufs=2)
                k_nxt = kpool.tile([P, S_], BF, tag="kT", name=f"k{t + 1}",
                                   bufs=2)
                thunks += proj_thunks(q_nxt, k_nxt, t + 1)
            # schedule thunk i after c-iteration floor(i * KT / len)
            sched = {}
            for i, th in enumerate(thunks):
                sched.setdefault(i * KT // max(1, len(thunks)), []).append(th)

            for c in range(KT):
                for n0, n1 in _chunks(SL_, 512):
                    w = n1 - n0
                    plt = plp.tile([P, 1024], F32, tag="pl",
                                   name=f"pl{ha}_{c}_{n0}", bufs=2)
                    nc.tensor.matmul(plt[:, 0:w],
                                     lhsT=k_cur[0:DEPTH, c * P:(c + 1) * P],
                                     rhs=q_cur[0:DEPTH, n0:n1],
                                     start=True, stop=True)
                    nc.tensor.matmul(plt[:, w:2 * w],
                                     lhsT=k_cur[DEPTH:2 * DEPTH,
                                                c * P:(c + 1) * P],
                                     rhs=q_cur[DEPTH:2 * DEPTH, n0:n1],
                                     start=True, stop=True)
                    ew = epool.tile([P, 1024], BF, tag="ew",
                                    name=f"ew{ha}_{c}_{n0}", bufs=2)
                    nc.scalar.activation(ew[:, 0:2 * w], plt[:, 0:2 * w], EXP)
                    ew2 = epool.tile([P, 1024], BF, tag="ew2",
                                     name=f"ew2{ha}_{c}_{n0}", bufs=2)
                    nc.vector.tensor_mul(ew2[:, 0:w], ew[:, 0:w],
                                         eb_sb[c][:, n0:n1])
                    nc.vector.tensor_mul(ew2[:, w:2 * w], ew[:, w:2 * w],
                                         eb_sb[c][:, n0:n1])
                    nc.tensor.matmul(pattn[0][:, n0:n1],
                                     lhsT=v_sb[c][:, ha, 0:65],
                                     rhs=ew2[:, 0:w],
                                     start=(c == 0), stop=(c == KT - 1))
                    nc.tensor.matmul(pattn[1][:, n0:n1],
                                     lhsT=v_sb[c][:, hb, 0:65],
                                     rhs=ew2[:, w:2 * w],
                                     start=(c == 0), stop=(c == KT - 1))
                for th in sched.get(c, ()):
                    th()

            # ---- epilogue + per-pair normalization ----
            # sau rows 0-63: unnormalized attn (base partition 0); row 64:
            # denominator.  Normalize in SBUF then bounce the finished rows
            # to DRAM so no [128,SL] attn tiles stay resident.
            den_t = smpool.tile([HPT, SL_], BF, tag="dent", name=f"den{t}",
                                bufs=2)
            saus = []
            for hf in range(HPT):
                h = ha + hf
                sau = stpool.tile([65, SL_], BF, tag="sau", name=f"sa{h}",
                                  bufs=3)
                saus.append(sau)
                nc.vector.tensor_copy(sau, pattn[hf])
                nc.sync.dma_start(out=den_t[hf:hf + 1, :], in_=sau[64:65, :])
            wj = min(512, SL_)
            jnk = None
            if t == NT - 1:
                jnk = plp.tile([1, 1024], F32, tag="pl", name="jnk", bufs=2)

            def beat(ap):
                # tiny dependent matmul: keeps the PE HAM activity window
                # alive across the serial normalize tail (else the output
                # projection starts at the 1.2 GHz throttled clock)
                if jnk is not None:
                    nc.tensor.matmul(jnk[0:1, 0:wj], lhsT=ap[0:1, 0:1],
                                     rhs=ap[0:1, 0:wj], start=True, stop=True)

            denf = smpool.tile([HPT, SL_], F32, tag="denf", name=f"dnf{t}",
                               bufs=1)
            nc.vector.tensor_copy(denf, den_t)
            beat(denf)
            recipf = smpool.tile([HPT, SL_], F32, tag="recipf",
                                 name=f"rcf{t}", bufs=1)
            nc.vector.reciprocal_approx_fast(recipf, denf)
            beat(recipf)
            recipb = smpool.tile([HPT, SL_], BF, tag="recipb",
                                 name=f"rcb{t}", bufs=1)
            nc.vector.tensor_copy(recipb, recipf)
            nc.sync.dma_start(out=rscr[HPT * t:HPT * (t + 1), :], in_=recipb)
            for hf in range(HPT):
                h = ha + hf
                bc = smpool.tile([DEPTH, SL_], BF, tag="bc", name=f"bc{h}",
                                 bufs=1)
                nc.sync.dma_start(
                    out=bc, in_=rscr[h:h + 1, :].partition_broadcast(DEPTH))
                beat(bc)
                anh = smpool.tile([DEPTH, SL_], BF, tag="anh", name=f"an{h}",
                                  bufs=2)
                nc.vector.tensor_mul(anh, saus[hf][0:64, :], bc)
                beat(anh)
                nc.sync.dma_start(
                    out=audram[t * P + hf * DEPTH:t * P + (hf + 1) * DEPTH, :],
                    in_=anh)
            if t + 1 < NT:
                q_cur, k_cur = q_nxt, k_nxt

    # ---------------- output projection ----------------
    opool = ctx.enter_context(tc.tile_pool(name="opool", bufs=2))
    wopool = ctx.enter_context(tc.tile_pool(name="wopool", bufs=NT))
    ropool = ctx.enter_context(tc.tile_pool(name="ropool", bufs=NT))
    wo_sb = [wopool.tile([P, D_], BF, tag="wo", name=f"wo{t}", bufs=NT)
             for t in range(NT)]
    for t in range(NT):
        nc.gpsimd.dma_start(out=wo_sb[t], in_=woT[t * P:(t + 1) * P, :])
    an_sb = [ropool.tile([P, SL_], BF, tag="an", name=f"ran{t}", bufs=NT)
             for t in range(NT)]
    for t in range(NT):
        nc.gpsimd.dma_start(out=an_sb[t], in_=audram[t * P:(t + 1) * P, :])
    jnk2 = plp.tile([1, 1024], F32, tag="pl", name="jnk2", bufs=2)
    wj = min(512, SL_)
    for t in (0, NT - 1):
        nc.tensor.matmul(jnk2[0:1, 0:wj], lhsT=an_sb[t][0:1, 0:1],
                         rhs=an_sb[t][0:1, 0:wj], start=True, stop=True)
    for m in range(NT):
        osb = opool.tile([P, SL_], F32, tag="osb", name=f"o{m}", bufs=2)
        for n0, n1 in _chunks(SL_, 512):
            ps = plp.tile([P, 1024], F32, tag="pl", name=f"pso{m}_{n0}", bufs=2)
            for t in range(NT):
                nc.tensor.matmul(ps[:, 0:n1 - n0],
                                 lhsT=wo_sb[t][:, m * P:(m + 1) * P],
                                 rhs=an_sb[t][:, n0:n1],
                                 start=(t == 0), stop=(t == NT - 1))
            nc.vector.tensor_copy(osb[:, n0:n1], ps[:, 0:n1 - n0])
        nc.sync.dma_start(out=outT[m * P:(m + 1) * P, :], in_=osb)


def build_nc(S_=S, D_=D, H_=H, SL_=None):
    if SL_ is None:
        SL_ = S_ // 2
    nc = bacc.Bacc("TRN2", target_bir_lowering=False, debug=False)
    io = {
        "xT": nc.dram_tensor("xT", [D_, SL_], BF, kind="ExternalInput").ap(),
        "yT": nc.dram_tensor("yT", [D_, S_], BF, kind="ExternalInput").ap(),
        "ebT": nc.dram_tensor("ebT", [S_, SL_], BF, kind="ExternalInput").ap(),
        "wqT": nc.dram_tensor("wqT", [D_, D_], BF, kind="ExternalInput").ap(),
        "wkT": nc.dram_tensor("wkT", [D_, D_], BF, kind="ExternalInput").ap(),
        "wvT": nc.dram_tensor("wvT", [D_, D_], BF, kind="ExternalInput").ap(),
        "woT": nc.dram_tensor("woT", [D_, D_], BF, kind="ExternalInput").ap(),
        "outT": nc.dram_tensor("outT", [D_, SL_], F32,
                               kind="ExternalOutput").ap(),
    }
    with tile.TileContext(nc) as tc:
        with ExitStack() as ctx:
            _attn_body(ctx, tc, io, S_, D_, H_, SL_)
    nc.compile()
    return nc


_NC_CACHE = None


def kernel(x, y, bias, Wq, Wk, Wv, Wo):
    global _NC_CACHE, last_exec_time_ns, last_results
    x = np.asarray(x, np.float32)
    y = np.asarray(y, np.float32)
    bias = np.asarray(bias, np.float32)
    Wq, Wk, Wv, Wo = (np.asarray(w, np.float32) for w in (Wq, Wk, Wv, Wo))
    SL_ = S // 2
    if _NC_CACHE is None:
        _NC_CACHE = build_nc()
    nc = _NC_CACHE

    bf = ml_dtypes.bfloat16
    scale = DEPTH ** -0.5
    wqT = np.ascontiguousarray(Wq.T * scale).astype(bf)
    wkT = np.ascontiguousarray(Wk.T).astype(bf)
    wvT = np.ascontiguousarray(Wv.T).astype(bf)
    woT = np.ascontiguousarray(Wo.T).astype(bf)
    eb = np.exp(bias[0, 0].astype(np.float32))
    ebT_half = [np.ascontiguousarray(eb[q0:q0 + SL_, :].T).astype(bf)
                for q0 in (0, SL_)]
    yT_all = [np.ascontiguousarray(y[b].T).astype(bf) for b in range(B)]

    in_maps = []
    for core in range(NCORES):
        b, half = divmod(core, 2)
        qs = half * SL_
        in_maps.append({
            "xT": np.ascontiguousarray(x[b, qs:qs + SL_, :].T).astype(bf),
            "yT": yT_all[b],
            "ebT": ebT_half[half],
            "wqT": wqT, "wkT": wkT, "wvT": wvT, "woT": woT,
        })

    res = run_bass_kernel_spmd(nc, in_maps, core_ids=list(range(NCORES)),
                               trace=TRACE)
    last_exec_time_ns = res.exec_time_ns
    last_results = res
    out = np.empty((B, S, D), np.float32)
    for core in range(NCORES):
        b, half = divmod(core, 2)
        qs = half * SL_
        out[b, qs:qs + SL_, :] = res.results[core]["outT"].T
    return out
